# revision 86
# baseline (speedup 1.0000x reference)
"""GQA with sliding-window + ALiBi (reduces to banded causal attention) on 8 TRN2 cores.

Sharding: 8 cores = 2 batches x 4 kv-head groups. Each core computes, for its
(batch b, kv group gi): Q projection for its 4 query heads, K/V projection for
its 1 kv head, banded sliding-window attention (window 1024, causal), and a
partial row-parallel Wo matmul. Host sums the 4 partials per batch.

Math notes (exact reductions of the reference):
- ALiBi bias is -clip(j-i,0)*slope: zero on all causal positions, nonzero only
  where the causal mask kills the score -> drop it entirely.
- The sliding mask adds +1.0 uniformly inside the window: softmax-invariant.
- Out-of-window/causal positions are exactly zeroed by multiplying exp(score)
  with a 0/1 mask (scores are O(1) so exp never overflows).
- Scores are O(1), so softmax without max-subtraction is safe.

Implementation notes (v1, bf16):
- All activations/weights stream as bf16 (halves DMA + SBUF; PE rate is the
  same 1 cycle/row as fp32r, accumulation stays fp32 in PSUM).
- Big batched DMAs (one per weight tensor / hsT chunk) to amortize the shared
  HWDGE descriptor stage.
- Attention is software-pipelined: score matmuls + exp run 3 quads ahead of
  the AV/denominator matmuls so PE never waits on the Act engine.
- Wo is interleaved one query-group behind attention, spreading its PSUM
  drain + output DMA across the attention phase.
- PSUM->SBUF drains are spread across DVE/Act/Pool engines.
"""
import math
from contextlib import ExitStack

import numpy as np
import ml_dtypes

import concourse.tile as tile
from concourse import bacc, mybir
from concourse.bass_utils import run_bass_kernel_spmd
from concourse.masks import make_identity

dt = mybir.dt
BF16 = ml_dtypes.bfloat16

B, S, H = 2, 2048, 2048
NUM_HEADS, KV_HEADS, D = 16, 4, 128
WINDOW = 1024
GH = 4            # query heads per kv head (per core)
GD = GH * D       # 512: per-core slice of the hidden dim
SCALE = 1.0 / math.sqrt(D)
QB = 256          # query columns per attention group
NG = S // QB      # 8 query groups
KT = H // 128     # 16 contraction tiles for projections

_nc_cache = None


def _build_nc(depth=3):
    nc = bacc.Bacc()
    hsT = nc.declare_dram_parameter("hsT", [4, KT, 128, 512], dt.bfloat16, isOutput=False)
    wqkv = nc.declare_dram_parameter("wqkv", [KT, 128, GD + 2 * D], dt.bfloat16, isOutput=False)
    wo = nc.declare_dram_parameter("wo", [4, 128, H], dt.bfloat16, isOutput=False)
    masks = nc.declare_dram_parameter("masks", [128, 768], dt.bfloat16, isOutput=False)
    out = nc.declare_dram_parameter("out", [16, 4, 128, 512], dt.bfloat16, isOutput=True)

    with tile.TileContext(nc) as tc, ExitStack() as ctx:
        consts = ctx.enter_context(tc.tile_pool(name="consts", bufs=1))
        wpool = ctx.enter_context(tc.tile_pool(name="wpool", bufs=1))
        big = ctx.enter_context(tc.tile_pool(name="big", bufs=1))
        hstp = ctx.enter_context(tc.tile_pool(name="hstp", bufs=2))
        vtp = ctx.enter_context(tc.tile_pool(name="vtp", bufs=2))
        ptp = ctx.enter_context(tc.tile_pool(name="ptp", bufs=6))
        smalls = ctx.enter_context(tc.tile_pool(name="smalls", bufs=5))
        outp = ctx.enter_context(tc.tile_pool(name="outp", bufs=4))

        # constants
        ident32 = consts.tile([128, 128], dt.float32)
        make_identity(nc, ident32)
        ident = consts.tile([128, 128], dt.float32r)
        nc.vector.tensor_copy(ident, ident32)
        ones32 = consts.tile([128, 128], dt.float32)
        nc.vector.memset(ones32, 1.0)
        ones_bf = consts.tile([128, 128], dt.bfloat16)
        nc.vector.tensor_copy(ones_bf, ones32)
        mask_t = consts.tile([128, 768], dt.bfloat16)

        # weights (single big SBUF tiles, loaded with few big DMAs)
        wqkv_sb = wpool.tile([128, KT * (GD + 2 * D)], dt.bfloat16)  # 24KB/part
        wo_sb = wpool.tile([128, 4 * H], dt.bfloat16)        # 16KB/part

        # persistent activations (bf16)
        qT = [big.tile([128, S], dt.bfloat16, name=f"qT{h}") for h in range(GH)]
        kT = big.tile([128, S], dt.bfloat16)
        v = big.tile([128, S], dt.bfloat16)   # [key, d] layout per 128-block
        ohT = [big.tile([128, S], dt.bfloat16, name=f"ohT{h}") for h in range(GH)]

        wqkv_v = wqkv_sb.rearrange("p (t n) -> p t n", t=KT)

        # ---- Phase 2 machinery (shared between the psA and psB PSUM pools) --
        mask_R = mask_t[:, 0:384]
        mask_L = mask_t[:, 384:768]
        pending = []
        fin_done = set()

        def drain(n):
            while len(pending) > n:
                pending.pop(0)()

        wo_parts = []  # deferred per-(st,e) Wo emission closures

        def emit_head(g, h, pstile):
            kjs = list(range(max(0, 2 * g - 8), 2 * g + 2))
            prs = [kjs[i:i + 2] for i in range(0, len(kjs), 2)]
            nb = len(prs)
            av = pstile([128, QB], "av", 2, f"av{h}_{g}")
            ptsum = None
            prev_pt = None
            for bi, pr in enumerate(prs):
                kind = 'R' if bi == nb - 1 else ('L' if bi == 0 and g >= 4 else 'P')
                # entries: (kj, col0, width, qoff); av order full-first
                if kind == 'R':       # [o=+1 right-half | o=0 full]
                    ents = [(2 * g, 256, 256, 0), (2 * g + 1, 128, 128, 128)]
                    erg = slice(128, 512)
                    zrg = slice(0, 128)
                elif kind == 'L':     # [o=-7 full | o=-8 left-half]
                    ents = [(2 * g - 7, 0, 256, 0), (2 * g - 8, 256, 128, 0)]
                    erg = slice(0, 384)
                    zrg = slice(384, 512)
                else:
                    ents = [(pr[0], 0, 256, 0), (pr[1], 256, 256, 0)]
                    erg = slice(0, 512)
                    zrg = None
                sps = pstile([128, 512], "sps", 3, f"sps{h}_{g}_{bi}")
                for kj, c0, w, qo in ents:
                    nc.tensor.matmul(
                        sps[:, c0:c0 + w],
                        lhsT=kT[:, kj * 128:(kj + 1) * 128],
                        rhs=qT[h][:, g * QB + qo:g * QB + qo + w],
                        start=True, stop=True)
                pt = ptp.tile([128, 512], dt.bfloat16, tag="pt",
                              name=f"pt{h}_{g}_{bi}")
                if zrg is not None:
                    nc.gpsimd.memset(pt[:, zrg], 0.0)
                nc.scalar.activation(
                    pt[:, erg], sps[:, erg],
                    mybir.ActivationFunctionType.Exp, scale=SCALE)
                if kind == 'R':
                    nc.vector.tensor_mul(pt[:, erg], pt[:, erg], mask_R)
                elif kind == 'L':
                    nc.vector.tensor_mul(pt[:, erg], pt[:, erg], mask_L)
                if bi == 1:
                    ptsum = smalls.tile([128, 512], dt.bfloat16, tag="ptsum",
                                        name=f"ptsum{h}_{g}")

                def mk_av(pt=pt, ents=ents, bi=bi, first=(bi == 0),
                          last=(bi == nb - 1), av=av, ptsum=ptsum,
                          prev_pt=prev_pt):
                    def f():
                        for i, (kj, c0, w, qo) in enumerate(ents):
                            nc.tensor.matmul(
                                av[:, qo:qo + w],
                                lhsT=v[:, kj * 128:(kj + 1) * 128],
                                rhs=pt[:, c0:c0 + w],
                                start=(first and i == 0),
                                stop=(last and i == len(ents) - 1))
                        # running pt-sum (softmax denominator); first
                        # add on the otherwise-idle Pool engine
                        if bi == 1:
                            nc.gpsimd.tensor_add(ptsum, prev_pt, pt)
                        elif bi > 1:
                            nc.vector.tensor_add(ptsum, ptsum, pt)
                    return f
                pending.append(mk_av())
                drain(depth)
                prev_pt = pt

            def mk_fin(h=h, g=g, av=av, ptsum=ptsum, pt=pt, pstile=pstile):
                def f():
                    # fold the two kj-halves -> per-q key-sums [128, 256]
                    src = ptsum if ptsum is not None else pt
                    ptf = smalls.tile([128, QB], dt.bfloat16, tag="ptf",
                                      name=f"ptf{h}_{g}")
                    nc.vector.tensor_add(ptf, src[:, 0:QB], src[:, QB:2 * QB])
                    denb = pstile([128, QB], "den", 1, f"den{h}_{g}")
                    nc.tensor.matmul(denb, lhsT=ones_bf, rhs=ptf,
                                     start=True, stop=True)
                    rcb = smalls.tile([128, QB], dt.float32r, tag="bcs",
                                      name=f"rcb{h}_{g}")
                    with nc.allow_low_precision(reason="f32r is full fp32 bits"):
                        nc.vector.reciprocal(rcb, denb)
                    nc.vector.tensor_mul(
                        ohT[h][:, g * QB:(g + 1) * QB], av, rcb)
                    fin_done.add((h, g))
                return f
            pending.append(mk_fin())

        # ---- Phase 1: projections (per 512-wide s-chunk) ----
        with tc.tile_pool(name="psA", bufs=8, space="PSUM") as psA:
            hst_tiles = []
            for ch in range(4):
                hst = hstp.tile([128, KT * 512], dt.bfloat16, tag="hst", name=f"hst{ch}")
                hst_tiles.append(hst)
            # chunk 0: quarter-granularity DMAs interleaved with weight quarters
            h0v = hst_tiles[0].rearrange("p (t n) -> p t n", t=KT)
            for sl in (slice(0, 1), slice(1, 2), slice(2, 4), slice(4, 7),
                       slice(7, 11), slice(11, 16)):
                nc.sync.dma_start(out=wqkv_v[:, sl], in_=wqkv[sl].rearrange("t p n -> p t n"))
                nc.sync.dma_start(out=h0v[:, sl], in_=hsT[0, sl].rearrange("t p n -> p t n"))

            for ch in range(4):
                if ch + 1 < 4:
                    nxt = hst_tiles[ch + 1]
                    nc.sync.dma_start(
                        out=nxt.rearrange("p (t n) -> p t n", t=KT),
                        in_=hsT[ch + 1].rearrange("t p n -> p t n"))
                if ch == 0:
                    nc.sync.dma_start(out=mask_t, in_=masks[:, :])
                    nc.sync.dma_start(
                        out=wo_sb.rearrange("p (c n) -> p c n", c=4),
                        in_=wo[:].rearrange("c p n -> p c n"))
                hst = hst_tiles[ch]
                q_ps = [psA.tile([128, 512], dt.float32, tag="ps", name=f"qps{ch}_{h}")
                        for h in range(GH)]
                k_ps = psA.tile([128, 512], dt.float32, tag="ps")
                v_ps = psA.tile([128, 512], dt.float32, tag="ps")

                def vtrans(ch, vt, pool=None):
                    # transpose V of a finished chunk, interleaved into the
                    # next chunk's matmul stream so PE never waits on it
                    for j in range(4):
                        if pool is None:
                            tp = psA.tile([128, 128], dt.float32r, tag="ps",
                                          name=f"tp{ch}_{j}")
                        else:
                            tp = pool.tile([128, 128], dt.float32r, tag="sps",
                                           bufs=3, name=f"tp{ch}_{j}")
                        nc.tensor.transpose(tp, vt[:, j * 128:(j + 1) * 128], ident)
                        nc.scalar.copy(
                            v[:, (4 * ch + j) * 128:(4 * ch + j + 1) * 128], tp)

                for t in range(KT):
                    if ch > 0 and t == 4:
                        vtrans(ch - 1, prev_vt)
                    rhs = hst[:, t * 512:(t + 1) * 512]
                    st = (t == 0)
                    sp = (t == KT - 1)
                    for h in range(GH):
                        nc.tensor.matmul(
                            q_ps[h], lhsT=wqkv_sb[:, t * 768 + h * 128: t * 768 + (h + 1) * 128],
                            rhs=rhs, start=st, stop=sp)
                    nc.tensor.matmul(k_ps, lhsT=wqkv_sb[:, t * 768 + 512: t * 768 + 640],
                                     rhs=rhs, start=st, stop=sp)
                    nc.tensor.matmul(v_ps, lhsT=wqkv_sb[:, t * 768 + 640: t * 768 + 768],
                                     rhs=rhs, start=st, stop=sp)
                # drain PSUM on three engines in slot-rotation order
                cs = slice(ch * 512, (ch + 1) * 512)
                nc.vector.tensor_copy(qT[0][:, cs], q_ps[0])
                nc.scalar.copy(qT[1][:, cs], q_ps[1])
                nc.vector.tensor_copy(qT[2][:, cs], q_ps[2])
                nc.vector.tensor_copy(qT[3][:, cs], q_ps[3])
                nc.scalar.copy(kT[:, cs], k_ps)
                vt = vtp.tile([128, 512], dt.float32r, tag="vt")
                nc.scalar.copy(vt, v_ps)
                prev_vt = vt

        # ---- Phase 2+3: banded attention (S^T[k,q] layout) + interleaved Wo ----
        with tc.tile_pool(name="psB", bufs=1, space="PSUM") as psB:
            def psb_tile(shape, tag, bufs, name):
                return psB.tile(shape, dt.float32, tag=tag, bufs=bufs, name=name)

            vtrans(3, prev_vt, pool=psB)

            def mk_wo(st, e, osb):
                def f():
                    wop = psB.tile([128, 512], dt.float32, tag="wop", bufs=2,
                                   name=f"wop{st}_{e}")
                    for ct in range(4):
                        nc.tensor.matmul(
                            wop, lhsT=ohT[ct][:, st * 128:(st + 1) * 128],
                            rhs=wo_sb[:, ct * 2048 + e * 512: ct * 2048 + (e + 1) * 512],
                            start=(ct == 0), stop=(ct == 3))
                    nc.scalar.copy(osb[:, e * 512:(e + 1) * 512], wop)
                    nc.sync.dma_start(
                        out=out[st, e], in_=osb[:, e * 512:(e + 1) * 512])
                return f

            def queue_wo(g):
                for st in (2 * g, 2 * g + 1):
                    osb = outp.tile([128, 2048], dt.bfloat16, tag="osb", name=f"osb{st}")
                    for e in range(4):
                        wo_parts.append(mk_wo(st, e, osb))

            for g in range(NG):
                if g >= 1:
                    queue_wo(g - 1)
                npop = 2
                for h in range(GH):
                    emit_head(g, h, psb_tile)
                    # interleave Wo pieces of earlier groups; their ohT
                    # inputs must have been written (fins emitted) first
                    if g >= 1:
                        while (3, g - 1) not in fin_done and pending:
                            pending.pop(0)()
                        for _ in range(npop):
                            if wo_parts:
                                wo_parts.pop(0)()
            drain(0)
            queue_wo(NG - 1)
            while wo_parts:
                wo_parts.pop(0)()

    nc.compile()
    return nc


def _build_masks():
    kk = np.arange(128)[:, None]
    qq = np.arange(256)[None, :]
    cc = np.arange(128)[None, :]
    # mask_R covers pt cols [128:512] of an R quad: [o=+1 right half | o=0 full]
    r1 = (kk <= cc).astype(np.float32)             # o = +1 on q in [128:256)
    r0 = (kk <= qq).astype(np.float32)             # o = 0
    # mask_L covers pt cols [0:384] of an L quad: [o=-7 full | o=-8 left half]
    l1 = (kk + 128 >= qq).astype(np.float32)       # o = -7
    l0 = (kk >= cc).astype(np.float32)             # o = -8 on q in [0:128)
    return np.hstack([r1, r0, l1, l0]).astype(BF16)  # [128, 768]


def kernel(hidden_states, Wq, Wk, Wv, Wo):
    global _nc_cache
    if _nc_cache is None:
        _nc_cache = _build_nc()
    nc = _nc_cache

    # accept jax or numpy inputs
    hidden_states = np.asarray(hidden_states, np.float32)
    Wq = np.asarray(Wq, np.float32)
    Wk = np.asarray(Wk, np.float32)
    Wv = np.asarray(Wv, np.float32)
    Wo = np.asarray(Wo, np.float32)

    masks = _build_masks()
    hsT = []
    for b in range(B):
        ht = np.ascontiguousarray(hidden_states[b].T)                 # [H, S]
        t4 = ht.reshape(KT, 128, 4, 512).transpose(2, 0, 1, 3)        # [ch, t, 128, 512]
        hsT.append(np.ascontiguousarray(t4).astype(BF16))
    in_maps = []
    for b in range(B):
        for gi in range(KV_HEADS):
            wqkv = np.concatenate(
                [Wq[:, gi * GD:(gi + 1) * GD], Wk[:, gi * D:(gi + 1) * D],
                 Wv[:, gi * D:(gi + 1) * D]], axis=1)
            in_maps.append({
                "hsT": hsT[b],
                "wqkv": wqkv.reshape(KT, 128, GD + 2 * D).astype(BF16),
                "wo": Wo[gi * GD:(gi + 1) * GD, :].reshape(4, 128, H).astype(BF16),
                "masks": masks,
            })
    res = run_bass_kernel_spmd(nc, in_maps, list(range(8)))
    out = np.zeros((B, S, H), np.float32)
    for b in range(B):
        acc = None
        for gi in range(KV_HEADS):
            o = np.asarray(res.results[b * KV_HEADS + gi]["out"]).astype(np.float32)
            acc = o if acc is None else acc + o
        out[b] = acc.transpose(0, 2, 1, 3).reshape(S, H)              # [16,4,128,512] -> [S,H]
    return out
